# revision 23
# baseline (speedup 1.0000x reference)
"""Trainium2 8-core Bass kernel for ContrastiveHGNN (hypergraph message passing).

Math (per layer l, behavior b, with Theta/behavior-weight folded into weights):
    yh_b  = Dv_b * (x @ Wt_bl + bt_bl)          # Wt_bl = W_node[l] @ Theta[b] * w[b]
    z_b   = H_b^T @ yh_b                        # edge gather  (needs AllReduce over node shards)
    u_b   = H_b @ (De_b * z_b)                  # node scatter
    x'    = relu(sum_b Dv_b * u_b)
Sharding: nodes split across 8 cores (512 rows each). Each core holds its
H row-slab [n,e] (fp8, exact for 0/1) for the gather and the matching
H^T col-slab [e,n] (fp8) for the scatter; activations in bf16.
One bf16 AllReduce of z (+ edge-degree columns, layer 0) per layer.
"""
import sys

sys.path.insert(0, "/opt/trn_rl_repo")

import numpy as np
import ml_dtypes

import concourse.bass as bass
import concourse.bacc as bacc
import concourse.tile as tile
import concourse.mybir as mybir
from concourse import bass_utils

dt = mybir.dt
BF16 = ml_dtypes.bfloat16
FP8 = ml_dtypes.float8_e4m3

NCORES = 8
N = 4096
E = 4096
B = 3
D_IN = 64
HID = 32
L = 2
OUT_D = 32
EPS = 1e-6

NLOC = N // NCORES          # 512 local nodes per core
NCH = NLOC // 128           # 4 row chunks of 128 local nodes
ET = E // 128               # 32 edge tiles/chunks of 128
ZW = HID + 1                # 33: z tile width (h + degree column)
ZPACK = 8                   # z tiles packed per PSUM bank tile
ZBLK = ZPACK * ZW           # 264 f32 <= 512 (one PSUM bank)
ZTOT = B * ET * ZW          # 3168 columns of z_all


def _build():
    nc = bacc.Bacc("TRN2", target_bir_lowering=False, debug=False,
                   num_devices=NCORES)

    # -------- DRAM I/O (per core) --------
    xt_d = nc.dram_tensor("xt", [D_IN + 1, NLOC], dt.bfloat16, kind="ExternalInput")
    hrow_d = nc.dram_tensor("hrow", [128, B * NCH * E], dt.float8e4, kind="ExternalInput")
    hcol_d = nc.dram_tensor("hcol", [128, B * ET * NLOC], dt.float8e4, kind="ExternalInput")
    w0_d = nc.dram_tensor("w0", [D_IN + 1, HID], dt.bfloat16, kind="ExternalInput")
    wl_d = nc.dram_tensor("wl", [HID + 1, L * B * HID], dt.bfloat16, kind="ExternalInput")
    wp_d = nc.dram_tensor("wp", [HID + 1, 2 * OUT_D], dt.bfloat16, kind="ExternalInput")
    ones_d = nc.dram_tensor("onesc", [128, 32], dt.float8e4, kind="ExternalInput")
    ssum_d = nc.dram_tensor("ssum", [B * HID, HID], dt.bfloat16, kind="ExternalInput")
    out_d = nc.dram_tensor("out", [NLOC, OUT_D], dt.float32, kind="ExternalOutput")

    with tile.TileContext(nc) as tc:
        with tc.tile_pool(name="big", bufs=1) as big, \
             tc.tile_pool(name="sb", bufs=1) as sb, \
             tc.tile_pool(name="zps_pool", bufs=3, space="PSUM") as zps_pool, \
             tc.tile_pool(name="ps1", bufs=1, space="PSUM") as ps1, \
             tc.tile_pool(name="ps2", bufs=1, space="PSUM") as ps2, \
             tc.tile_pool(name="dram", bufs=1, space="DRAM") as dram:

            # -------- resident SBUF tensors --------
            hcol = big.tile([128, B * ET * NLOC], dt.float8e4)
            hrow = big.tile([128, B * NCH * E], dt.float8e4)
            xt = sb.tile([D_IN + 1, NLOC], dt.bfloat16)
            w0 = sb.tile([D_IN + 1, HID], dt.bfloat16)
            wl = sb.tile([HID + 1, L * B * HID], dt.bfloat16)
            wp = sb.tile([HID + 1, 2 * OUT_D], dt.bfloat16)
            onesc = sb.tile([128, 32], dt.float8e4)
            ssum = sb.tile([B * HID, HID], dt.bfloat16)
            nc.gpsimd.dma_start(ssum[:], ssum_d[:])

            nc.gpsimd.dma_start(xt[:], xt_d[:])
            nc.gpsimd.dma_start(w0[:], w0_d[:])
            nc.gpsimd.dma_start(wl[:], wl_d[:])
            nc.gpsimd.dma_start(wp[:], wp_d[:])
            nc.gpsimd.dma_start(onesc[:], ones_d[:])
            # stream H slabs behavior-interleaved on two independent DMA
            # paths so degv (hcol) and z-step (hrow) can chase per behavior
            HCB = 4 * NLOC  # 2048 cols = 4 edge-chunk blocks
            for b in range(B):
                for k in range(8 * b, 8 * (b + 1)):
                    nc.sync.dma_start(hcol[:, k * HCB:(k + 1) * HCB],
                                      hcol_d[:, k * HCB:(k + 1) * HCB])
                for k in range(NCH * b, NCH * (b + 1)):
                    nc.scalar.dma_start(hrow[:, k * E:(k + 1) * E],
                                        hrow_d[:, k * E:(k + 1) * E])

            # -------- initial transform: x0T = relu(W0^T @ Xt) --------
            # lhsT = w0 [65, 32], rhs = xt [65, 512] -> psum [32, 512]
            x0ps = ps2.tile([HID, NLOC], dt.float32, tag="wide")
            nc.tensor.matmul(x0ps[:], w0[:], xt[:], start=True, stop=True)
            xtl = sb.tile([HID + 1, NLOC], dt.bfloat16)   # x^T with ones row
            nc.gpsimd.memset(xtl[:], 1.0)
            nc.vector.tensor_scalar_max(xtl[0:HID, :], x0ps[:], 0.0)

            # -------- node degrees: deg_v[n] = sum_e H[n, e] --------
            # DoubleRow fp8 ones-matmul over the col-slab: contracts 256
            # edges per MM, stationary loaded once -> [1, n] output layout
            ones3 = onesc[:].rearrange("p (a c) -> p a c", a=2)[:, :, 0:1]
            dvT = sb.tile([1, B * NLOC], dt.float32)
            for b in range(B):
                degps = ps1.tile([1, NLOC], dt.float32, tag="degv", bufs=3,
                                 name=f"degps{b}")
                for ecp in range(ET // 2):
                    rhs = hcol[:, (b * ET + 2 * ecp) * NLOC:
                               (b * ET + 2 * ecp + 2) * NLOC].rearrange(
                        "p (a n) -> p a n", a=2)
                    nc.tensor.matmul(degps[:], ones3, rhs,
                                     start=(ecp == 0), stop=(ecp == ET // 2 - 1),
                                     perf_mode=mybir.MatmulPerfMode.DoubleRow)
                nc.vector.tensor_scalar_max(
                    dvT[:, b * NLOC:(b + 1) * NLOC], degps[:], EPS)
            # dv = 1/sqrt(max(deg, eps)) in [1, (b, n)] layout
            nc.vector.reciprocal(dvT[:], dvT[:])
            nc.scalar.activation(dvT[:], dvT[:],
                                 mybir.ActivationFunctionType.Sqrt)
            dvd = dram.tile([B, NLOC], dt.float32)
            nc.sync.dma_start(dvd[:].rearrange("b n -> (b n)")[None, :], dvT[:])
            # dv_part [128, (b, nch)] for the per-partition yh scaling
            dv_part = sb.tile([128, B * NCH], dt.float32)
            src = bass.AP(dvd.tensor, dvd.offset, [[1, 128], [NLOC, B], [128, NCH]])
            nc.sync.dma_start(dv_part[:], src)
            # dv_bc [3*32, n] for the u-step epilogue (free-axis layout)
            dv_bc = sb.tile([B * HID, NLOC], dt.float32)
            for b in range(B):
                bsrc = bass.AP(dvd.tensor, dvd.offset + b * NLOC,
                               [[0, HID], [1, NLOC]])
                nc.sync.dma_start(dv_bc[b * HID:(b + 1) * HID, :], bsrc)

            # -------- per-layer tensors --------
            yh = sb.tile([128, B * NCH * ZW], dt.bfloat16)
            nc.gpsimd.memset(yh[:], 1.0)  # ones column (col 32 of each block)

            xw_ps = ps1.tile([128, B * NCH * HID], dt.float32, tag="small")
            zcast = sb.tile([128, ZTOT], dt.bfloat16)
            z_all = sb.tile([128, ZTOT], dt.bfloat16)
            de_sb = sb.tile([128, B * ET], dt.float32)
            zs = sb.tile([128, B * ET * HID], dt.bfloat16)

            for layer in range(L):
                # ---- yh_b = dv * (x W~_bl + b~_bl), plus ones col ----
                for b in range(B):
                    for nch in range(NCH):
                        blk = b * NCH + nch
                        nc.tensor.matmul(
                            xw_ps[:, blk * HID:(blk + 1) * HID],
                            xtl[:, nch * 128:(nch + 1) * 128],
                            wl[:, (layer * B + b) * HID:(layer * B + b + 1) * HID],
                            start=True, stop=True)
                        nc.vector.tensor_scalar(
                            yh[:, blk * ZW:blk * ZW + HID],
                            xw_ps[:, blk * HID:(blk + 1) * HID],
                            dv_part[:, blk:blk + 1], None,
                            mybir.AluOpType.mult)

                # ---- z-step: z_b[e_tile] = sum_nch Hrow_b,nch^T @ yh_b,nch ----
                cc_in = dram.tile([128, ZTOT], dt.bfloat16, name=f"cc_in{layer}")
                cc_out = dram.tile([128, ZTOT], dt.bfloat16, name=f"cc_out{layer}",
                                   addr_space="Shared")
                ztile_idx = 0
                for b in range(B):
                    for tg in range(ET // ZPACK):   # groups of 8 e-tiles
                        zps = zps_pool.tile([128, ZBLK], dt.float32, tag="z")
                        for nch in range(NCH):
                            for ti in range(ZPACK):
                                t = tg * ZPACK + ti
                                nc.tensor.matmul(
                                    zps[:, ti * ZW:(ti + 1) * ZW],
                                    hrow[:, (b * NCH + nch) * E + t * 128:
                                         (b * NCH + nch) * E + t * 128 + 128],
                                    yh[:, (b * NCH + nch) * ZW:
                                       (b * NCH + nch) * ZW + ZW],
                                    start=(nch == 0), stop=(nch == NCH - 1))
                        sl = slice(ztile_idx * ZBLK, (ztile_idx + 1) * ZBLK)
                        nc.vector.tensor_copy(zcast[:, sl], zps[:])
                        # ship each cast block to the AR input as it lands
                        nc.sync.dma_start(cc_in[:, sl], zcast[:, sl])
                        ztile_idx += 1

                # ---- AllReduce z over the 8 node shards (bf16 wire) ----
                nc.gpsimd.collective_compute(
                    "AllReduce", mybir.AluOpType.add,
                    replica_groups=[list(range(NCORES))],
                    ins=[cc_in[:].opt()], outs=[cc_out[:].opt()])

                # ---- return + de/zs scaling in quarters so the u-step can
                #      start on early edge chunks while later ones stream ----
                NQ = 4
                QT = ET // NQ                      # 8 e-tiles per quarter
                for q in range(NQ):
                    src = cc_out[:].rearrange("p (b r) -> p b r", b=B)[
                        :, :, q * QT * ZW:(q + 1) * QT * ZW]
                    dst = z_all[:].rearrange("p (b r) -> p b r", b=B)[
                        :, :, q * QT * ZW:(q + 1) * QT * ZW]
                    nc.sync.dma_start(dst, src)
                    z3 = z_all[:].rearrange("p (b t w) -> p b t w", w=ZW, t=ET)
                    if layer == 0:
                        de3 = de_sb[:].rearrange("p (b t) -> p b t", b=B)
                        nc.vector.tensor_scalar_max(
                            de3[:, :, q * QT:(q + 1) * QT, None],
                            z3[:, :, q * QT:(q + 1) * QT, HID:HID + 1], EPS)
                        nc.vector.reciprocal(de3[:, :, q * QT:(q + 1) * QT],
                                             de3[:, :, q * QT:(q + 1) * QT])
                    zg = z3[:, :, q * QT:(q + 1) * QT, 0:HID]
                    zsg = zs[:].rearrange("p (b t w) -> p b t w", w=HID, t=ET)[
                        :, :, q * QT:(q + 1) * QT, :]
                    de4 = de_sb[:].rearrange("p (b t) -> p b t", b=B)[
                        :, :, q * QT:(q + 1) * QT, None]
                    nc.vector.tensor_tensor(
                        zsg, zg, de4.broadcast_to([128, B, QT, HID]),
                        mybir.AluOpType.mult)

                # ---- u-step: aggT_b = sum_ec zs_b[ec]^T @ Hcol_b[ec] ----
                # col-tiled: behavior b -> PE column group b, psum rows 32b:32b+32
                agg_ps = ps2.tile([B * HID, NLOC], dt.float32, tag="wide")
                for ec in range(ET):
                    for b in range(B):
                        nc.tensor.matmul(
                            agg_ps[b * HID:(b + 1) * HID, :],
                            zs[:, (b * ET + ec) * HID:(b * ET + ec + 1) * HID],
                            hcol[:, (b * ET + ec) * NLOC:(b * ET + ec + 1) * NLOC],
                            start=(ec == 0), stop=(ec == ET - 1),
                            tile_position=(0, b * HID))

                # ---- x' = relu(sum_b dv_b * aggT_b); append ones row ----
                # behavior-sum across partition groups via selection matmul
                usc = sb.tile([B * HID, NLOC], dt.bfloat16, name=f"usc{layer}")
                nc.vector.tensor_tensor(usc[:], agg_ps[:], dv_bc[:],
                                        mybir.AluOpType.mult)
                xs_ps = ps2.tile([HID, NLOC], dt.float32, tag="wide",
                                 name=f"xs_ps{layer}")
                nc.tensor.matmul(xs_ps[:], ssum[:], usc[:], start=True, stop=True)
                xtl = sb.tile([HID + 1, NLOC], dt.bfloat16, name=f"xtl{layer}")
                nc.gpsimd.memset(xtl[:], 1.0)
                nc.vector.tensor_scalar_max(xtl[0:HID, :], xs_ps[:], 0.0)

            # -------- projection: relu(x Wp1 + bp1) Wp2 + bp2 --------
            p1ps = ps2.tile([OUT_D, NLOC], dt.float32, tag="wide")
            nc.tensor.matmul(p1ps[:], wp[:, 0:OUT_D], xtl[:], start=True, stop=True)
            p1t = sb.tile([OUT_D + 1, NLOC], dt.bfloat16)
            nc.gpsimd.memset(p1t[:], 1.0)
            nc.vector.tensor_scalar_max(p1t[0:OUT_D, :], p1ps[:], 0.0)
            out_ps = ps1.tile([128, NCH * OUT_D], dt.float32, tag="small")
            out_sb = sb.tile([128, NCH * OUT_D], dt.float32)
            for nch in range(NCH):
                nc.tensor.matmul(
                    out_ps[:, nch * OUT_D:(nch + 1) * OUT_D],
                    p1t[:, nch * 128:(nch + 1) * 128],
                    wp[:, OUT_D:2 * OUT_D], start=True, stop=True)
                nc.vector.tensor_copy(out_sb[:, nch * OUT_D:(nch + 1) * OUT_D],
                                      out_ps[:, nch * OUT_D:(nch + 1) * OUT_D])
                nc.sync.dma_start(out_d[nch * 128:(nch + 1) * 128, :],
                                  out_sb[:, nch * OUT_D:(nch + 1) * OUT_D])

    nc.compile()
    return nc


_NC_CACHE = None


def _get_nc():
    global _NC_CACHE
    if _NC_CACHE is None:
        _NC_CACHE = _build()
    return _NC_CACHE


def _prep_inputs(X, H, W_init, b_init, W_node, b_node, Theta,
                 behavior_importance, Wp1, bp1, Wp2, bp2):
    f32 = np.float32
    X = np.asarray(X, f32)
    H = np.asarray(H, f32)
    W_init = np.asarray(W_init, f32)
    b_init = np.asarray(b_init, f32)
    W_node = np.asarray(W_node, f32)
    b_node = np.asarray(b_node, f32)
    Theta = np.asarray(Theta, f32)
    bi = np.asarray(behavior_importance, f32)
    Wp1 = np.asarray(Wp1, f32)
    bp1 = np.asarray(bp1, f32)
    Wp2 = np.asarray(Wp2, f32)
    bp2 = np.asarray(bp2, f32)

    w = np.exp(bi - bi.max())
    w = w / w.sum()                                   # softmax behavior weights

    # fold Theta & behavior weight into per-layer node weights
    wl = np.zeros((HID + 1, L * B * HID), f32)
    for l in range(L):
        for b in range(B):
            Wt = (W_node[l] @ Theta[b]) * w[b]        # [32, 32]
            bt = (b_node[l] @ Theta[b]) * w[b]        # [32]
            wl[0:HID, (l * B + b) * HID:(l * B + b + 1) * HID] = Wt
            wl[HID, (l * B + b) * HID:(l * B + b + 1) * HID] = bt
    w0 = np.concatenate([W_init, b_init[None, :]], 0)   # [65, 32]
    wp = np.zeros((HID + 1, 2 * OUT_D), f32)
    wp[0:HID, 0:OUT_D] = Wp1
    wp[HID, 0:OUT_D] = bp1
    wp[0:HID, OUT_D:] = Wp2
    wp[HID, OUT_D:] = bp2

    w0_bf = w0.astype(BF16)
    wl_bf = wl.astype(BF16)
    wp_bf = wp.astype(BF16)
    ones_fp8 = np.ones((128, 32), FP8)
    ssum = np.zeros((B * HID, HID), f32)
    for b in range(B):
        ssum[b * HID:(b + 1) * HID, :] = np.eye(HID, dtype=f32)
    ssum_bf = ssum.astype(BF16)

    Xt1 = np.concatenate([X.T, np.ones((1, N), f32)], 0).astype(BF16)  # [65, 4096]

    H8 = (H != 0).astype(FP8)                          # exact 0/1 in fp8

    in_maps = []
    for c in range(NCORES):
        lo, hi = c * NLOC, (c + 1) * NLOC
        # row-slab: [128, (b, nch) x E]; [p, (b*NCH+nch)*E + e] = H[b, lo+nch*128+p, e]
        hrow = np.ascontiguousarray(
            H8[:, lo:hi, :].reshape(B, NCH, 128, E)
            .transpose(2, 0, 1, 3).reshape(128, B * NCH * E))
        # col-slab: [128, (b, ec) x NLOC]; [p, (b*ET+ec)*NLOC + j] = H[b, lo+j, ec*128+p]
        hcol = np.ascontiguousarray(
            H8[:, lo:hi, :].transpose(0, 2, 1).reshape(B, ET, 128, NLOC)
            .transpose(2, 0, 1, 3).reshape(128, B * ET * NLOC))
        in_maps.append({
            "xt": np.ascontiguousarray(Xt1[:, lo:hi]),
            "hrow": hrow,
            "hcol": hcol,
            "w0": w0_bf,
            "wl": wl_bf,
            "wp": wp_bf,
            "onesc": ones_fp8,
            "ssum": ssum_bf,
        })
    return in_maps


def kernel(**inputs):
    nc = _get_nc()
    in_maps = _prep_inputs(**inputs)
    res = bass_utils.run_bass_kernel_spmd(nc, in_maps, core_ids=list(range(NCORES)))
    out = np.concatenate([np.asarray(res.results[c]["out"], np.float32)
                          for c in range(NCORES)], 0)
    return out


# revision 30
# speedup vs baseline: 1.1665x; 1.1665x over previous
"""Trainium2 8-core Bass kernel for ContrastiveHGNN (hypergraph message passing).

Math (per layer l, behavior b, with Theta/behavior-weight folded into weights):
    yh_b  = Dv_b * (x @ Wt_bl + bt_bl)          # Wt_bl = W_node[l] @ Theta[b] * w[b]
    z_b   = H_b^T @ yh_b                        # edge gather  (needs AllReduce over node shards)
    u_b   = H_b @ (De_b * z_b)                  # node scatter
    x'    = relu(sum_b Dv_b * u_b)
Sharding: nodes split across 8 cores (512 rows each). Each core holds its
H row-slab [n,e] (fp8, exact for 0/1) for the gather and the matching
H^T col-slab [e,n] (fp8) for the scatter; activations in bf16.
One bf16 AllReduce of z (+ edge-degree columns, layer 0) per layer.
"""
import sys

sys.path.insert(0, "/opt/trn_rl_repo")

import numpy as np
import ml_dtypes

import concourse.bass as bass
import concourse.bacc as bacc
import concourse.tile as tile
import concourse.mybir as mybir
from concourse import bass_utils

dt = mybir.dt
BF16 = ml_dtypes.bfloat16
FP8 = ml_dtypes.float8_e4m3

NCORES = 8
N = 4096
E = 4096
B = 3
D_IN = 64
HID = 32
L = 2
OUT_D = 32
EPS = 1e-6

NLOC = N // NCORES          # 512 local nodes per core
NCH = NLOC // 128           # 4 row chunks of 128 local nodes
ET = E // 128               # 32 edge tiles/chunks of 128
ZW = HID + 1                # 33: z tile width (h + degree column)
ZPACK = 8                   # z tiles packed per PSUM bank tile
ZBLK = ZPACK * ZW           # 264 f32 <= 512 (one PSUM bank)
ZTOT = B * ET * ZW          # 3168 columns of z_all


def _build():
    nc = bacc.Bacc("TRN2", target_bir_lowering=False, debug=False,
                   num_devices=NCORES)

    # -------- DRAM I/O (per core) --------
    xt_d = nc.dram_tensor("xt", [D_IN + 1, NLOC], dt.bfloat16, kind="ExternalInput")
    hrow_d = nc.dram_tensor("hrow", [128, B * NCH * E], dt.float8e4, kind="ExternalInput")
    hcol_d = nc.dram_tensor("hcol", [128, B * ET * NLOC], dt.float8e4, kind="ExternalInput")
    w0_d = nc.dram_tensor("w0", [D_IN + 1, HID], dt.bfloat16, kind="ExternalInput")
    wl_d = nc.dram_tensor("wl", [HID + 1, L * B * HID], dt.bfloat16, kind="ExternalInput")
    wp_d = nc.dram_tensor("wp", [HID + 1, 2 * OUT_D], dt.bfloat16, kind="ExternalInput")
    ones_d = nc.dram_tensor("onesc", [128, 32], dt.float8e4, kind="ExternalInput")
    ssum_d = nc.dram_tensor("ssum", [B * HID, HID], dt.bfloat16, kind="ExternalInput")
    ident_d = nc.dram_tensor("ident", [128, 128], dt.float32, kind="ExternalInput")
    out_d = nc.dram_tensor("out", [NLOC, OUT_D], dt.float32, kind="ExternalOutput")

    with tile.TileContext(nc) as tc:
        with tc.tile_pool(name="big", bufs=1) as big, \
             tc.tile_pool(name="sb", bufs=1) as sb, \
             tc.tile_pool(name="zps_pool", bufs=6, space="PSUM") as zps_pool, \
             tc.tile_pool(name="ps1", bufs=1, space="PSUM") as ps1, \
             tc.tile_pool(name="ps2", bufs=1, space="PSUM") as ps2, \
             tc.tile_pool(name="dram", bufs=1, space="DRAM") as dram:

            # -------- resident SBUF tensors --------
            hcol = big.tile([128, B * ET * NLOC], dt.float8e4)
            hrow = big.tile([128, B * NCH * E], dt.float8e4)
            xt = sb.tile([D_IN + 1, NLOC], dt.bfloat16)
            w0 = sb.tile([D_IN + 1, HID], dt.bfloat16)
            wl = sb.tile([HID + 1, L * B * HID], dt.bfloat16)
            wp = sb.tile([HID + 1, 2 * OUT_D], dt.bfloat16)
            onesc = sb.tile([128, 32], dt.float8e4)
            ssum = sb.tile([B * HID, HID], dt.bfloat16)
            ident = sb.tile([128, 128], dt.float32)
            nc.gpsimd.dma_start(ssum[:], ssum_d[:])
            nc.gpsimd.dma_start(ident[:], ident_d[:])

            nc.gpsimd.dma_start(xt[:], xt_d[:])
            nc.gpsimd.dma_start(w0[:], w0_d[:])
            nc.gpsimd.dma_start(wl[:], wl_d[:])
            nc.gpsimd.dma_start(wp[:], wp_d[:])
            nc.gpsimd.dma_start(onesc[:], ones_d[:])
            # hcol first at full bandwidth (degv chases it on the PE), then
            # hrow chunks land just in time for the chasing z-step
            HCB = 4 * NLOC  # 2048 cols = 4 edge-chunk blocks
            for k in range(B * ET * NLOC // HCB):
                nc.sync.dma_start(hcol[:, k * HCB:(k + 1) * HCB],
                                  hcol_d[:, k * HCB:(k + 1) * HCB])
            for k in range(B * NCH):
                nc.sync.dma_start(hrow[:, k * E:(k + 1) * E],
                                  hrow_d[:, k * E:(k + 1) * E])

            # -------- initial transform: x0T = relu(W0^T @ Xt) --------
            # lhsT = w0 [65, 32], rhs = xt [65, 512] -> psum [32, 512]
            x0ps = ps2.tile([HID, NLOC], dt.float32, tag="wide")
            nc.tensor.matmul(x0ps[:], w0[:], xt[:], start=True, stop=True)
            xtl = sb.tile([HID + 1, NLOC], dt.bfloat16)   # x^T with ones row
            nc.gpsimd.memset(xtl[:], 1.0)
            nc.vector.tensor_scalar_max(xtl[0:HID, :], x0ps[:], 0.0)

            # -------- node degrees: deg_v[n] = sum_e H[n, e] --------
            # lhsT = hcol block [e=128, n=128] fp8, rhs = ones col [128, 1]
            degps = ps1.tile([128, B * NCH], dt.float32, tag="small")
            for b in range(B):
                for nch in range(NCH):
                    col = b * NCH + nch
                    for ec in range(ET):
                        nc.tensor.matmul(
                            degps[:, col:col + 1],
                            hcol[:, (b * ET + ec) * NLOC + nch * 128:
                                 (b * ET + ec) * NLOC + nch * 128 + 128],
                            onesc[:, 0:1],
                            start=(ec == 0), stop=(ec == ET - 1))
            # dv = 1/sqrt(max(deg, eps))  [128, (b, nch)] f32 partition-major
            dv_part = sb.tile([128, B * NCH], dt.float32)
            nc.vector.tensor_scalar_max(dv_part[:], degps[:], EPS)
            nc.vector.reciprocal(dv_part[:], dv_part[:])
            nc.scalar.activation(dv_part[:], dv_part[:],
                                 mybir.ActivationFunctionType.Sqrt)
            # epilogue broadcast layout: PE-transpose -> DRAM (contiguous
            # rows) -> stride-0 broadcast reads to [3*32, n]
            dvt_ps = ps1.tile([B * NCH, 128], dt.float32, tag="small")
            nc.tensor.transpose(dvt_ps[:], dv_part[:], ident[:])
            dvt_sb = sb.tile([B * NCH, 128], dt.float32)
            nc.vector.tensor_copy(dvt_sb[:], dvt_ps[:])
            dvd = dram.tile([B * NCH, 128], dt.float32)
            nc.sync.dma_start(dvd[:], dvt_sb[:])
            dv_bc = sb.tile([B * HID, NLOC], dt.float32)
            for b in range(B):
                bsrc = bass.AP(dvd.tensor, dvd.offset + b * NLOC,
                               [[0, HID], [1, NLOC]])
                nc.sync.dma_start(dv_bc[b * HID:(b + 1) * HID, :], bsrc)

            # -------- per-layer tensors --------
            yh = sb.tile([128, B * NCH * ZW], dt.bfloat16)
            nc.gpsimd.memset(yh[:], 1.0)  # ones column (col 32 of each block)

            xw_ps = ps1.tile([128, B * NCH * HID], dt.float32, tag="small")
            zcast = sb.tile([128, ZTOT], dt.bfloat16)
            z_all = sb.tile([128, ZTOT], dt.bfloat16)
            de_sb = sb.tile([128, B * ET], dt.float32)
            zs = sb.tile([128, B * ET * HID], dt.bfloat16)

            for layer in range(L):
                # ---- yh_b = dv * (x W~_bl + b~_bl), plus ones col ----
                for b in range(B):
                    for nch in range(NCH):
                        blk = b * NCH + nch
                        nc.tensor.matmul(
                            xw_ps[:, blk * HID:(blk + 1) * HID],
                            xtl[:, nch * 128:(nch + 1) * 128],
                            wl[:, (layer * B + b) * HID:(layer * B + b + 1) * HID],
                            start=True, stop=True)
                        nc.vector.tensor_scalar(
                            yh[:, blk * ZW:blk * ZW + HID],
                            xw_ps[:, blk * HID:(blk + 1) * HID],
                            dv_part[:, blk:blk + 1], None,
                            mybir.AluOpType.mult)

                # ---- z-step: z_b[e_tile] = sum_nch Hrow_b,nch^T @ yh_b,nch ----
                cc_in = dram.tile([128, ZTOT], dt.bfloat16, name=f"cc_in{layer}")
                cc_out = dram.tile([128, ZTOT], dt.bfloat16, name=f"cc_out{layer}",
                                   addr_space="Shared")
                ztile_idx = 0
                for b in range(B):
                    for tg in range(ET // ZPACK):   # groups of 8 e-tiles
                        zps = zps_pool.tile([128, ZBLK], dt.float32, tag="z")
                        for nch in range(NCH):
                            for ti in range(ZPACK):
                                t = tg * ZPACK + ti
                                nc.tensor.matmul(
                                    zps[:, ti * ZW:(ti + 1) * ZW],
                                    hrow[:, (b * NCH + nch) * E + t * 128:
                                         (b * NCH + nch) * E + t * 128 + 128],
                                    yh[:, (b * NCH + nch) * ZW:
                                       (b * NCH + nch) * ZW + ZW],
                                    start=(nch == 0), stop=(nch == NCH - 1))
                        sl = slice(ztile_idx * ZBLK, (ztile_idx + 1) * ZBLK)
                        nc.vector.tensor_copy(zcast[:, sl], zps[:])
                        # ship each cast block to the AR input as it lands
                        nc.scalar.dma_start(cc_in[:, sl], zcast[:, sl])
                        ztile_idx += 1

                # ---- AllReduce z over the 8 node shards (bf16 wire) ----
                nc.gpsimd.collective_compute(
                    "AllReduce", mybir.AluOpType.add,
                    replica_groups=[list(range(NCORES))],
                    ins=[cc_in[:].opt()], outs=[cc_out[:].opt()])

                # ---- return + de/zs scaling in quarters so the u-step can
                #      start on early edge chunks while later ones stream ----
                NQ = 4
                QT = ET // NQ                      # 8 e-tiles per quarter
                for q in range(NQ):
                    src = cc_out[:].rearrange("p (b r) -> p b r", b=B)[
                        :, :, q * QT * ZW:(q + 1) * QT * ZW]
                    dst = z_all[:].rearrange("p (b r) -> p b r", b=B)[
                        :, :, q * QT * ZW:(q + 1) * QT * ZW]
                    nc.sync.dma_start(dst, src)
                    z3 = z_all[:].rearrange("p (b t w) -> p b t w", w=ZW, t=ET)
                    if layer == 0:
                        de3 = de_sb[:].rearrange("p (b t) -> p b t", b=B)
                        nc.vector.tensor_scalar_max(
                            de3[:, :, q * QT:(q + 1) * QT, None],
                            z3[:, :, q * QT:(q + 1) * QT, HID:HID + 1], EPS)
                        nc.vector.reciprocal(de3[:, :, q * QT:(q + 1) * QT],
                                             de3[:, :, q * QT:(q + 1) * QT])
                    zg = z3[:, :, q * QT:(q + 1) * QT, 0:HID]
                    zsg = zs[:].rearrange("p (b t w) -> p b t w", w=HID, t=ET)[
                        :, :, q * QT:(q + 1) * QT, :]
                    de4 = de_sb[:].rearrange("p (b t) -> p b t", b=B)[
                        :, :, q * QT:(q + 1) * QT, None]
                    nc.vector.tensor_tensor(
                        zsg, zg, de4.broadcast_to([128, B, QT, HID]),
                        mybir.AluOpType.mult)

                # ---- u-step: aggT_b = sum_ec zs_b[ec]^T @ Hcol_b[ec] ----
                # col-tiled: behavior b -> PE column group b, psum rows 32b:32b+32
                agg_ps = ps2.tile([B * HID, NLOC], dt.float32, tag="wide")
                for ec in range(ET):
                    for b in range(B):
                        nc.tensor.matmul(
                            agg_ps[b * HID:(b + 1) * HID, :],
                            zs[:, (b * ET + ec) * HID:(b * ET + ec + 1) * HID],
                            hcol[:, (b * ET + ec) * NLOC:(b * ET + ec + 1) * NLOC],
                            start=(ec == 0), stop=(ec == ET - 1),
                            tile_position=(0, b * HID))

                # ---- x' = relu(sum_b dv_b * aggT_b); append ones row ----
                # behavior-sum across partition groups via selection matmul
                usc = sb.tile([B * HID, NLOC], dt.bfloat16, name=f"usc{layer}")
                nc.vector.tensor_tensor(usc[:], agg_ps[:], dv_bc[:],
                                        mybir.AluOpType.mult)
                xs_ps = ps2.tile([HID, NLOC], dt.float32, tag="wide",
                                 name=f"xs_ps{layer}")
                nc.tensor.matmul(xs_ps[:], ssum[:], usc[:], start=True, stop=True)
                xtl = sb.tile([HID + 1, NLOC], dt.bfloat16, name=f"xtl{layer}")
                nc.gpsimd.memset(xtl[:], 1.0)
                nc.vector.tensor_scalar_max(xtl[0:HID, :], xs_ps[:], 0.0)

            # -------- projection: relu(x Wp1 + bp1) Wp2 + bp2 --------
            p1ps = ps2.tile([OUT_D, NLOC], dt.float32, tag="wide")
            nc.tensor.matmul(p1ps[:], wp[:, 0:OUT_D], xtl[:], start=True, stop=True)
            p1t = sb.tile([OUT_D + 1, NLOC], dt.bfloat16)
            nc.gpsimd.memset(p1t[:], 1.0)
            nc.vector.tensor_scalar_max(p1t[0:OUT_D, :], p1ps[:], 0.0)
            out_ps = ps1.tile([128, NCH * OUT_D], dt.float32, tag="small")
            out_sb = sb.tile([128, NCH * OUT_D], dt.float32)
            for nch in range(NCH):
                nc.tensor.matmul(
                    out_ps[:, nch * OUT_D:(nch + 1) * OUT_D],
                    p1t[:, nch * 128:(nch + 1) * 128],
                    wp[:, OUT_D:2 * OUT_D], start=True, stop=True)
                nc.vector.tensor_copy(out_sb[:, nch * OUT_D:(nch + 1) * OUT_D],
                                      out_ps[:, nch * OUT_D:(nch + 1) * OUT_D])
                nc.sync.dma_start(out_d[nch * 128:(nch + 1) * 128, :],
                                  out_sb[:, nch * OUT_D:(nch + 1) * OUT_D])

    nc.compile()
    return nc


_NC_CACHE = None


def _get_nc():
    global _NC_CACHE
    if _NC_CACHE is None:
        _NC_CACHE = _build()
    return _NC_CACHE


def _prep_inputs(X, H, W_init, b_init, W_node, b_node, Theta,
                 behavior_importance, Wp1, bp1, Wp2, bp2):
    f32 = np.float32
    X = np.asarray(X, f32)
    H = np.asarray(H, f32)
    W_init = np.asarray(W_init, f32)
    b_init = np.asarray(b_init, f32)
    W_node = np.asarray(W_node, f32)
    b_node = np.asarray(b_node, f32)
    Theta = np.asarray(Theta, f32)
    bi = np.asarray(behavior_importance, f32)
    Wp1 = np.asarray(Wp1, f32)
    bp1 = np.asarray(bp1, f32)
    Wp2 = np.asarray(Wp2, f32)
    bp2 = np.asarray(bp2, f32)

    w = np.exp(bi - bi.max())
    w = w / w.sum()                                   # softmax behavior weights

    # fold Theta & behavior weight into per-layer node weights
    wl = np.zeros((HID + 1, L * B * HID), f32)
    for l in range(L):
        for b in range(B):
            Wt = (W_node[l] @ Theta[b]) * w[b]        # [32, 32]
            bt = (b_node[l] @ Theta[b]) * w[b]        # [32]
            wl[0:HID, (l * B + b) * HID:(l * B + b + 1) * HID] = Wt
            wl[HID, (l * B + b) * HID:(l * B + b + 1) * HID] = bt
    w0 = np.concatenate([W_init, b_init[None, :]], 0)   # [65, 32]
    wp = np.zeros((HID + 1, 2 * OUT_D), f32)
    wp[0:HID, 0:OUT_D] = Wp1
    wp[HID, 0:OUT_D] = bp1
    wp[0:HID, OUT_D:] = Wp2
    wp[HID, OUT_D:] = bp2

    w0_bf = w0.astype(BF16)
    wl_bf = wl.astype(BF16)
    wp_bf = wp.astype(BF16)
    ones_fp8 = np.ones((128, 32), FP8)
    ident = np.eye(128, dtype=f32)
    ssum = np.zeros((B * HID, HID), f32)
    for b in range(B):
        ssum[b * HID:(b + 1) * HID, :] = np.eye(HID, dtype=f32)
    ssum_bf = ssum.astype(BF16)

    Xt1 = np.concatenate([X.T, np.ones((1, N), f32)], 0).astype(BF16)  # [65, 4096]

    H8 = (H != 0).astype(FP8)                          # exact 0/1 in fp8

    in_maps = []
    for c in range(NCORES):
        lo, hi = c * NLOC, (c + 1) * NLOC
        # row-slab: [128, (b, nch) x E]; [p, (b*NCH+nch)*E + e] = H[b, lo+nch*128+p, e]
        hrow = np.ascontiguousarray(
            H8[:, lo:hi, :].reshape(B, NCH, 128, E)
            .transpose(2, 0, 1, 3).reshape(128, B * NCH * E))
        # col-slab: [128, (b, ec) x NLOC]; [p, (b*ET+ec)*NLOC + j] = H[b, lo+j, ec*128+p]
        hcol = np.ascontiguousarray(
            H8[:, lo:hi, :].transpose(0, 2, 1).reshape(B, ET, 128, NLOC)
            .transpose(2, 0, 1, 3).reshape(128, B * ET * NLOC))
        in_maps.append({
            "xt": np.ascontiguousarray(Xt1[:, lo:hi]),
            "hrow": hrow,
            "hcol": hcol,
            "w0": w0_bf,
            "wl": wl_bf,
            "wp": wp_bf,
            "onesc": ones_fp8,
            "ssum": ssum_bf,
            "ident": ident,
        })
    return in_maps


def kernel(**inputs):
    nc = _get_nc()
    in_maps = _prep_inputs(**inputs)
    res = bass_utils.run_bass_kernel_spmd(nc, in_maps, core_ids=list(range(NCORES)))
    out = np.concatenate([np.asarray(res.results[c]["out"], np.float32)
                          for c in range(NCORES)], 0)
    return out
